# revision 1
# baseline (speedup 1.0000x reference)
"""Chamfer distance kernel for Trainium2 (Bass/Tile), 8-core SPMD.

Problem: recon/target [64, 4, 2048] f32, mask [64, 2048] i32 ->
scalar mean chamfer loss (squared distances, masked min both directions).

Strategy (data-parallel over batch, 8 samples/core), v3: mask compaction.

The mask keeps ~50% of the 2048 points.  The host compacts each sample's
valid points to the front and pads to PC (= max valid count, ~1080)
columns with a far-point sentinel (16,16,16,16) whose squared distance
(>= ~480) can never win the min against any valid point, so padded
columns need no BIG masking and padded rows are simply dropped on the
host.  This shrinks each sample's pairwise matrix from 2048^2 to ~1080^2
(~3.6x less device work).

Device program per core (8 samples, 2 chamfer orientations):
  - Host pre-assembles bf16 operand tensors (error-free hi/lo split:
    dot = xhi.yhi + xhi.ylo + xlo.yhi, dropped xlo.ylo ~ 2^-16) with the
    halved negated column-norm vector -(yn)/2 riding as two extra K rows
    (ones x cvh/cvl), K=14 per 32-partition sample slot:
        V[n, m] = x_n . y_m - yn[m]/2
    so rowmax_m V = (xn[n] - d2min[n])/2 and the host recovers
    d2min = xn - 2*rowmax (clamped at 0) -- the per-row norm is applied
    post-hoc on the host, which keeps K small and needs no BIG terms.
  - Per 128-row block: cols [0:1024) go to a 2-bank PSUM tile (bufs=3 for
    a depth-3 pipeline); ScalarE stages cols [512:1024) to SBUF; one
    VectorE custom MAX2_REDUCE (out=max(in0,in1), accum=row-max) absorbs
    both halves, emitting the block's row max over cols [0:1024).
  - Cols [1024:PC) of four consecutive blocks land in 128-col slots of a
    shared 1-bank PSUM tile; one strided VectorE tensor_reduce absorbs
    all four slots into four per-block accum columns (the host maxes the
    two accum streams).
  - PE p-state: the cost model only reaches the 2.4 GHz p-state when the
    PE stays continuously busy, so small "warm" matmuls into a dead slot
    of the remainder bank pad every gap (warmup stretch + a few per
    block); they touch only constant SBUF tiles and dead PSUM columns.
  - Epilogue: DMA the row-max accum tiles to HBM; the host does
    relu/masking/means in numpy (O(B*N), negligible).
"""

import sys

import numpy as np

for _p in ("/opt/trn_rl_repo",):
    if _p not in sys.path:
        sys.path.append(_p)

B, F, N = 64, 4, 2048
N_CORES = 8
SPC = B // N_CORES  # samples per core
FAR = 16.0          # far-point sentinel coordinate
NEG_INIT = -3.0e38
GROUP = 4           # blocks sharing one remainder PSUM bank
NWARM = 80          # warmup dummy matmuls (PE p-state ramp, under DMA-in)
NDUMMY = 8          # per-block dummy matmuls bridging PE stalls

_CACHE = {}


def _register_max2_reduce():
    """Author + register a custom DVE op: out = max(in0, in1),
    accum_out = max-reduce(out) seeded from s0.  Absorbs two tiles per pass
    with the row-max fused."""
    from concourse import dve_ops
    from concourse.dve_spec import Spec, Src0, Src1, C0, maxx, lower, _has_src1
    from concourse.dve_uop import DveOpSpec

    NAME = "MAX2_REDUCE_ANT"
    for op in dve_ops.OPS:
        if op.name == NAME:
            return op

    def _ref_max2(in0, in1, c0, c1, c2):
        b = np.maximum(in0.astype(np.float32), in1.astype(np.float32))
        a = np.maximum(b.reshape(b.shape[0], -1).max(axis=-1, keepdims=True), c0)
        return b, a

    spec = Spec(body=maxx(Src0, Src1), accum=maxx, accum_init=C0,
                reference=_ref_max2)
    row = dve_ops._CUSTOM_DVE_ROW_BASE + len(dve_ops.OPS)
    shas = {}
    for ver in ("v3", "v4"):
        s = DveOpSpec(name=NAME, opcode=row, uops=lower(spec, ver=ver),
                      rd1_en=_has_src1(spec))
        shas[ver] = s.sha(ver)
    op = dve_ops.DveOp(NAME, spec, subdim=False, uops_sha=shas)
    dve_ops.OPS.append(op)
    dve_ops._SUB_OPCODE_FOR_NAME[NAME] = row
    dve_ops.CUSTOM_DVE_SPECS[NAME] = spec
    return op


def _build_bass(pr, pc):
    """Build the per-core program.

    pr: padded row count (multiple of 128; lhsT point columns)
    pc: padded column count (1024 < pc <= 1024+128, even), or pc == 1024
    """
    from contextlib import ExitStack

    import concourse.mybir as mybir
    import concourse.tile as tile
    from concourse import bacc

    f32 = mybir.dt.float32
    bf16 = mybir.dt.bfloat16
    Alu = mybir.AluOpType
    Axis = mybir.AxisListType

    max2 = _register_max2_reduce()

    nb = pr // 128
    W = pc - 1024  # remainder columns per block (0 <= W <= 128)
    ntiles = 2 * SPC * nb

    nc = bacc.Bacc("TRN2", target_bir_lowering=False, debug=False,
                   num_devices=N_CORES)

    L_dram = [[nc.dram_tensor(f"L{o}{g}", (128, pr), bf16,
                              kind="ExternalInput").ap()
               for g in range(2)] for o in range(2)]
    R_dram = [[nc.dram_tensor(f"R{o}{g}", (128, pc), bf16,
                              kind="ExternalInput").ap()
               for g in range(2)] for o in range(2)]
    nm_out = [nc.dram_tensor(f"nm{o}", (128, SPC * nb), f32,
                             kind="ExternalOutput").ap() for o in range(2)]
    if W > 0:
        nmr_out = nc.dram_tensor("nmr", (128, ntiles), f32,
                                 kind="ExternalOutput").ap()
        # per-sample lhsT/rhs copies at partitions 0:14 -- the remainder
        # matmuls write at sub-bank PSUM offsets, which hard-faults unless
        # tile_position is (0, 0)
        Lz_dram = [[nc.dram_tensor(f"Lz{o}{j}", (14, pr), bf16,
                                   kind="ExternalInput").ap()
                    for j in range(SPC)] for o in range(2)]
        Rz_dram = [[nc.dram_tensor(f"Rz{o}{j}", (14, W), bf16,
                                   kind="ExternalInput").ap()
                    for j in range(SPC)] for o in range(2)]

    with tile.TileContext(nc) as tc, ExitStack() as ctx:
        consts = ctx.enter_context(tc.tile_pool(name="consts", bufs=1))
        opnds = ctx.enter_context(tc.tile_pool(name="opnds", bufs=1))
        accum = ctx.enter_context(tc.tile_pool(name="accum", bufs=1))

        # dummy operands for warm matmuls (no DMA dependency)
        wl = consts.tile([14, 128], bf16, name="wl")
        wr = consts.tile([14, 64], bf16, name="wr")
        nc.gpsimd.memset(wl, 0.0)
        nc.gpsimd.memset(wr, 0.0)

        L_sb = [[opnds.tile([128, pr], bf16, tag=f"L{o}{g}", name=f"L{o}{g}")
                 for g in range(2)] for o in range(2)]
        R_sb = [[opnds.tile([128, pc], bf16, tag=f"R{o}{g}", name=f"R{o}{g}")
                 for g in range(2)] for o in range(2)]
        negmax = [accum.tile([128, SPC * nb], f32, tag=f"nm{o}", name=f"nm{o}")
                  for o in range(2)]
        if W > 0:
            negr = accum.tile([128, ntiles], f32, tag="nmr", name="nmr")
            Lz_sb = [[opnds.tile([14, pr], bf16, tag=f"Lz{o}{j}",
                                 name=f"Lz{o}{j}") for j in range(SPC)]
                     for o in range(2)]
            Rz_sb = [[opnds.tile([14, W], bf16, tag=f"Rz{o}{j}",
                                 name=f"Rz{o}{j}") for j in range(SPC)]
                     for o in range(2)]

        for o in range(2):
            for g in range(2):
                nc.sync.dma_start(out=L_sb[o][g], in_=L_dram[o][g])
                nc.sync.dma_start(out=R_sb[o][g], in_=R_dram[o][g])
            if W > 0:
                for j in range(SPC):
                    nc.sync.dma_start(out=Lz_sb[o][j], in_=Lz_dram[o][j])
                    nc.sync.dma_start(out=Rz_sb[o][j], in_=Rz_dram[o][j])

        with tc.tile_pool(name="stage", bufs=6) as stage, \
                tc.tile_pool(name="mm_ps", bufs=3, space="PSUM") as mm_ps, \
                tc.tile_pool(name="rem_ps", bufs=2, space="PSUM") as rem_ps:

            remt = rem_ps.tile([128, 512], f32, tag="rem", name="rem0")

            def dummy_mm(n):
                for _ in range(n):
                    nc.tensor.matmul(remt[:, 448:512], wl, wr,
                                     start=True, stop=True,
                                     tile_position=(0, 0))

            # p-state warmup while the operand DMAs land
            dummy_mm(NWARM)

            t = 0
            for o in range(2):
                for g in range(2):
                    for s in range(4):
                        j = g * 4 + s
                        p0 = 32 * s
                        Lt = L_sb[o][g]
                        Rt = R_sb[o][g]
                        for i in range(nb):
                            k = t % GROUP
                            if W > 0 and k == 0 and t > 0:
                                remt = rem_ps.tile([128, 512], f32, tag="rem",
                                                   name=f"rem{t}")
                            lhs = Lt[p0:p0 + 14, i * 128:(i + 1) * 128]
                            ps = mm_ps.tile([128, 1024], f32, tag="ps")
                            nc.tensor.matmul(ps[:, 512:1024], lhs,
                                             Rt[p0:p0 + 14, 512:1024],
                                             start=True, stop=True,
                                             tile_position=(p0, 0))
                            if W > 0:
                                nc.tensor.matmul(
                                    remt[:, 128 * k:128 * k + W],
                                    Lz_sb[o][j][:, i * 128:(i + 1) * 128],
                                    Rz_sb[o][j],
                                    start=True, stop=True,
                                    tile_position=(0, 0))
                            nc.tensor.matmul(ps[:, 0:512], lhs,
                                             Rt[p0:p0 + 14, 0:512],
                                             start=True, stop=True,
                                             tile_position=(p0, 0))
                            dummy_mm(NDUMMY)

                            st = stage.tile([128, 512], f32, tag="st")
                            nc.scalar.copy(st, ps[:, 512:1024])
                            junk = stage.tile([128, 512], f32, tag="junk")
                            nc.vector._custom_dve(
                                max2, out=junk, in0=ps[:, 0:512], in1=st,
                                s0=NEG_INIT,
                                accum_out=negmax[o][:, j * nb + i:
                                                    j * nb + i + 1])
                            if W > 0 and k == GROUP - 1:
                                nc.vector.tensor_reduce(
                                    negr[:, t - 3:t + 1],
                                    remt.rearrange(
                                        "p (k c) -> p k c", k=4)[:, :, 0:W],
                                    Axis.X, Alu.max)
                            t += 1

        with tc.tile_pool(name="ep", bufs=1):
            for o in range(2):
                nc.sync.dma_start(out=nm_out[o], in_=negmax[o])
            if W > 0:
                nc.sync.dma_start(out=nmr_out, in_=negr)

    nc.compile()
    return nc


def _get_nc(pr, pc):
    key = ("nc", pr, pc)
    if key not in _CACHE:
        _CACHE[key] = _build_bass(pr, pc)
    return _CACHE[key]


def _prep_core(recon_c, target_c, mask_c, pr, pc):
    """Build the operand tensors for one core + host-side leftovers."""
    import ml_dtypes

    bf16 = ml_dtypes.bfloat16
    L = [[np.zeros((128, pr), dtype=bf16) for _ in range(2)] for _ in range(2)]
    R = [[np.zeros((128, pc), dtype=bf16) for _ in range(2)] for _ in range(2)]
    post = []

    for j in range(SPC):
        m = mask_c[j] != 0
        cnt = int(m.sum())
        pad = max(pr, pc)
        xp = np.full((F, pad), FAR, dtype=np.float32)
        yp = np.full((F, pad), FAR, dtype=np.float32)
        xp[:, :cnt] = recon_c[j][:, m]
        yp[:, :cnt] = target_c[j][:, m]
        xn = np.sum(xp * xp, axis=0)  # [pad]
        yn = np.sum(yp * yp, axis=0)

        xh = xp.astype(bf16)
        xl = (xp - xh.astype(np.float32)).astype(bf16)
        yh = yp.astype(bf16)
        yl = (yp - yh.astype(np.float32)).astype(bf16)
        cvy = (-0.5 * yn).astype(np.float32)
        cvyh = cvy.astype(bf16)
        cvyl = (cvy - cvyh.astype(np.float32)).astype(bf16)
        cvx = (-0.5 * xn).astype(np.float32)
        cvxh = cvx.astype(bf16)
        cvxl = (cvx - cvxh.astype(np.float32)).astype(bf16)

        g, s = j // 4, j % 4
        p0 = 32 * s
        one = np.ones((pad,), dtype=bf16)
        for o in range(2):
            dh, dl = (xh, xl) if o == 0 else (yh, yl)      # lhsT data
            rh, rl = (yh, yl) if o == 0 else (xh, xl)      # rhs data
            ch, cl = (cvyh, cvyl) if o == 0 else (cvxh, cvxl)
            Lt, Rt = L[o][g], R[o][g]
            Lt[p0 + 0:p0 + 4] = dh[:, :pr]
            Lt[p0 + 4:p0 + 8] = dh[:, :pr]
            Lt[p0 + 8:p0 + 12] = dl[:, :pr]
            Lt[p0 + 12] = one[:pr]
            Lt[p0 + 13] = one[:pr]
            Rt[p0 + 0:p0 + 4] = rh[:, :pc]
            Rt[p0 + 4:p0 + 8] = rl[:, :pc]
            Rt[p0 + 8:p0 + 12] = rh[:, :pc]
            Rt[p0 + 12] = ch[:pc]
            Rt[p0 + 13] = cl[:pc]
        post.append((cnt, xn[:cnt].astype(np.float64),
                     yn[:cnt].astype(np.float64)))

    in_map = {}
    for o in range(2):
        for g in range(2):
            in_map[f"L{o}{g}"] = L[o][g]
            in_map[f"R{o}{g}"] = R[o][g]
    W = pc - 1024
    if W > 0:
        for o in range(2):
            for j in range(SPC):
                g, s = j // 4, j % 4
                p0 = 32 * s
                in_map[f"Lz{o}{j}"] = np.ascontiguousarray(
                    L[o][g][p0:p0 + 14, :])
                in_map[f"Rz{o}{j}"] = np.ascontiguousarray(
                    R[o][g][p0:p0 + 14, 1024:pc])
    return in_map, post


def kernel(recon, target, mask):
    recon = np.ascontiguousarray(recon, dtype=np.float32)
    target = np.ascontiguousarray(target, dtype=np.float32)
    mask_b = np.asarray(mask) != 0

    cnts = mask_b.sum(axis=1)
    cmax = int(cnts.max())
    if cmax <= 1024:
        pr, pc = 1024, 1024
    elif cmax <= 1152:
        pr, pc = 1152, min(1152, cmax + (cmax & 1))
    else:
        pr = pc = 2048
    nb = pr // 128
    nc = _get_nc(pr, pc)

    from concourse.bass_utils import run_bass_kernel_spmd

    in_maps = []
    posts = []
    for c in range(N_CORES):
        sl = slice(c * SPC, (c + 1) * SPC)
        im, post = _prep_core(recon[sl], target[sl], mask_b[sl], pr, pc)
        in_maps.append(im)
        posts.append(post)

    res = run_bass_kernel_spmd(nc, in_maps, core_ids=list(range(N_CORES)))

    W = pc - 1024
    loss_sum = 0.0
    for c in range(N_CORES):
        nm = [np.asarray(res.results[c][f"nm{o}"], dtype=np.float64)
              for o in range(2)]
        nmr = (np.asarray(res.results[c]["nmr"], dtype=np.float64)
               if W > 0 else None)
        for j in range(SPC):
            cnt, xn, yn = posts[c][j]
            g, s = j // 4, j % 4
            per = 0.0
            for o in range(2):
                vs = nm[o][:, j * nb:(j + 1) * nb]  # [128, nb]
                if W > 0:
                    t0 = ((o * 2 + g) * 4 + s) * nb
                    vr = nmr[:, t0:t0 + nb]
                    vs = np.maximum(vs, vr)
                vflat = vs.T.reshape(-1)  # point n = 128*i + r
                norms = xn if o == 0 else yn
                d2 = norms - 2.0 * vflat[:cnt]
                per += float(np.maximum(d2, 0.0).sum()) / cnt
            loss_sum += per
    loss = loss_sum / B
    return np.array(loss, dtype=np.float32)

